# revision 22
# baseline (speedup 1.0000x reference)
"""Causal self-attention block (RMSNorm + QKV + RoPE + causal attention +
out-proj + residual) on 8 Trainium2 NeuronCores.

Sharding: batch (B=2) x head-groups (16 heads -> 4 groups of 4) = 8 shards.
Core c handles batch b = c // 4 and heads [4*(c%4), 4*(c%4)+4).  The host
sums the 4 partial out-projections per batch and adds the residual.

v2 design (vs the row-major v1):
 - RMSNorm is folded host-side into the activations (h = x * rinv), and
   norm_w into w_qkv, so the device sees pre-normalized hT and does no
   stats / scaling work at all.
 - Q/K/V are projected DIRECTLY in transposed layout (dh on partitions,
   t free) by making the weight block the stationary operand, eliminating
   all PE transposes of v1.
 - RoPE pairs (i, i+32) are host-interleaved to adjacent partitions so
   rotate-half becomes a single DVE stream_shuffle (mask swaps even/odd
   partitions within each 32-partition quadrant).
 - fp8(e4m3) + MatmulPerfMode.DoubleRow (2 rows/cycle) for the QKV
   projection, PV, and out-projection matmuls; scores stay bf16 (exp
   input precision).  Weights are pre-scaled by WS=32 host-side to clear
   the fp8 denormal range; 1/WS is folded into the RoPE tables / evicts.
 - All inputs are host-packed into their exact SBUF layouts so each DMA
   moves large contiguous rows (128 descriptors per tensor).
 - scores^T (k on partitions) + ones-column in V give softmax denominators
   from the PV matmul; no max-subtraction needed (scores are O(3)).
"""

import numpy as np

import ml_dtypes

import concourse.bacc as bacc
import concourse.tile as tile
from concourse import mybir
from concourse.bass_utils import run_bass_kernel_spmd

# Problem shapes (hardcoded per contract)
B, T, D, NHEADS = 2, 2048, 1024, 16
HEAD_DIM = 64
EPS = 1e-6
ROPE_BASE = 10000.0

HL = 4            # heads per core
P = 128
NT = T // P       # 16 t-tiles
NQC = T // 512    # 4 query chunks
NCORES = 8
WS = 32.0         # host-side weight pre-scale (fp8 denormal avoidance)
SC = 0.125        # 1/sqrt(64)

F32 = mybir.dt.float32
BF16 = mybir.dt.bfloat16
FP8 = mybir.dt.float8e4
DR = mybir.MatmulPerfMode.DoubleRow

# w8 column map (fp8 constants, packed host-side in exact SBUF layout)
W8_QK = 0          # 4096 cols: [m(4) x j(8) x c(128)]
W8_V = 4096        # 2048 cols: [j(8) x c(256)]
W8_WO = 6144       # 2048 cols: [jb(2) x e(1024)]
W8_TRI = 8192      # 128 cols
W8_M256 = 8320     # 256 cols: [zeros(128) | tri] for odd diagonal members
W8_COLS = 8576

CS_COLS = 4096     # cos (2048) | sin (2048), bf16

# stream-shuffle mask: swap even/odd partitions within each 32-quadrant
SHUF = [i ^ 1 for i in range(32)]


def _build_program():
    nc = bacc.Bacc("TRN2", target_bir_lowering=False, debug=False,
                   num_devices=NCORES)

    w8 = nc.dram_tensor("w8", [P, W8_COLS], FP8, kind="ExternalInput").ap()
    cs = nc.dram_tensor("cs", [P, CS_COLS], BF16, kind="ExternalInput").ap()
    ht = nc.dram_tensor("ht", [P, NQC * 4096], FP8, kind="ExternalInput").ap()
    outp = nc.dram_tensor("outp", [T, D], BF16, kind="ExternalOutput").ap()
    dbg = {}
    if DEBUG:
        dbg["d_qT0"] = nc.dram_tensor("d_qT0", [P, 1024], BF16,
                                      kind="ExternalOutput").ap()
        dbg["d_kT"] = nc.dram_tensor("d_kT", [P, NQC * 1024], BF16,
                                     kind="ExternalOutput").ap()
        dbg["d_v0"] = nc.dram_tensor("d_v0", [P, 768], FP8,
                                     kind="ExternalOutput").ap()
        dbg["d_att0"] = nc.dram_tensor("d_att0", [P, 1024], FP8,
                                       kind="ExternalOutput").ap()
        dbg["d_pt00"] = nc.dram_tensor("d_pt00", [P, 1024], FP8,
                                       kind="ExternalOutput").ap()

    with tile.TileContext(nc) as tc:
        _emit(tc, w8, cs, ht, outp, dbg)

    nc.compile()
    return nc


DEBUG = False


def _emit(tc, w8, cs, ht, outp, dbg=None):
    nc = tc.nc
    from contextlib import ExitStack
    ctx = ExitStack()
    with ctx:
        const = ctx.enter_context(tc.tile_pool(name="const", bufs=1))
        persist = ctx.enter_context(tc.tile_pool(name="persist", bufs=1))
        qtp = ctx.enter_context(tc.tile_pool(name="qtp", bufs=2))
        atp = ctx.enter_context(tc.tile_pool(name="atp", bufs=2))
        ptp = ctx.enter_context(tc.tile_pool(name="ptp", bufs=20))
        shp = ctx.enter_context(tc.tile_pool(name="shp", bufs=3))
        s2p = ctx.enter_context(tc.tile_pool(name="s2p", bufs=3))
        nrm = ctx.enter_context(tc.tile_pool(name="nrm", bufs=4))
        orow = ctx.enter_context(tc.tile_pool(name="orow", bufs=3))
        # PSUM budget (8 banks): qk 2 + sm 2x2 + pvop 2
        psp = ctx.enter_context(
            tc.tile_pool(name="psp", bufs=2, space="PSUM"))

        # ---- SBUF-resident inputs (host-packed layouts) ----
        # split across the two HWDGE queues (sync + scalar) so the first
        # projection's operands land as early as possible
        w8_sb = const.tile([P, W8_COLS], FP8)
        for m in range(4):
            nc.sync.dma_start(out=w8_sb[:, 1024 * m:1024 * (m + 1)],
                              in_=w8[:, 1024 * m:1024 * (m + 1)])
        ht_sb = persist.tile([P, NQC * 4096], FP8)
        nc.scalar.dma_start(out=ht_sb[:, 0:2048], in_=ht[:, 0:2048])
        nc.sync.dma_start(out=ht_sb[:, 2048:4096], in_=ht[:, 2048:4096])
        cs_sb = const.tile([P, CS_COLS], BF16)
        nc.sync.dma_start(out=w8_sb[:, 4096:W8_COLS], in_=w8[:, 4096:W8_COLS])
        nc.scalar.dma_start(out=cs_sb[:], in_=cs[:])
        for qc in range(1, NQC):
            eng = nc.sync if qc % 2 else nc.scalar
            eng.dma_start(out=ht_sb[:, 4096 * qc:4096 * (qc + 1)],
                          in_=ht[:, 4096 * qc:4096 * (qc + 1)])

        wqk = w8_sb[:, W8_QK:W8_QK + 4096].rearrange(
            "p (m j c) -> p m j c", m=4, c=P)
        wv = w8_sb[:, W8_V:W8_V + 2048].rearrange("p (j c) -> p j c", c=256)
        wo = w8_sb[:, W8_WO:W8_WO + 2048].rearrange("p (j e) -> p j e", e=D)
        tri = w8_sb[:, W8_TRI:W8_TRI + P]
        m256 = w8_sb[:, W8_M256:W8_M256 + 256]
        htr = ht_sb[:].rearrange("p (q j t) -> p q j t", q=NQC, t=512)

        # K^T persistent: chunk qc block b (heads 2b,2b+1) at cols
        # [1024*qc + 512*b]; partitions = RoPE-interleaved dh of 2 heads.
        kT = persist.tile([P, NQC * 1024], BF16)
        # V row-major pair tiles: pair kp = k-tiles (2kp, 2kp+1); layout
        # [p, member(2) x head(4) x c(96)]; c=64 is the ones column and
        # c=65..95 zero padding (dual-fp8 ldweights needs M % 32 == 0).
        v_t = [persist.tile([P, 768], FP8, name=f"v{i}", tag=f"v{i}")
               for i in range(NT // 2)]
        for kp in range(NT // 2):
            vr = v_t[kp].rearrange("p (m h c) -> p m h c", m=2, c=96)
            nc.vector.memset(vr[:, :, :, HEAD_DIM:HEAD_DIM + 1], 1.0)
            nc.vector.memset(vr[:, :, :, HEAD_DIM + 1:], 0.0)

        def rope_evict(ps, dst, qc):
            """dst = ps*cos + shuffle(ps)*sin, all (128, 512); 1/WS folded
            into the host tables."""
            ct = cs_sb[:, 512 * qc:512 * (qc + 1)]
            st = cs_sb[:, 2048 + 512 * qc:2048 + 512 * (qc + 1)]
            t1 = shp.tile([P, 512], F32, tag="t1")
            nc.vector.stream_shuffle(t1[:], ps, SHUF)
            t2 = s2p.tile([P, 512], BF16, tag="t2")
            nc.vector.tensor_mul(t2[:], t1[:], st)
            nc.vector.tensor_mul(dst, ps, ct)
            nc.vector.tensor_add(dst, dst, t2[:])

        def proj_qk(qc, part, qT=None):
            """Transposed-domain Q/K projection + RoPE for one 512-chunk.
            part 0 = the two q blocks (allocates qT); part 1 = k blocks."""
            if part == 0:
                qT = qtp.tile([P, 1024], BF16, tag="qT")
            for m in (0, 1) if part == 0 else (2, 3):
                ps = psp.tile([P, 512], F32, tag="qk")
                for jp in range(4):
                    nc.tensor.matmul(
                        ps[:], wqk[:, m, 2 * jp:2 * jp + 2, :],
                        htr[:, qc, 2 * jp:2 * jp + 2, :],
                        start=(jp == 0), stop=(jp == 3), perf_mode=DR)
                if m < 2:
                    dst = qT[:, 512 * m:512 * (m + 1)]
                else:
                    dst = kT[:, 1024 * qc + 512 * (m - 2):
                             1024 * qc + 512 * (m - 1)]
                rope_evict(ps[:], dst, qc)
            return qT

        def proj_v(qc):
            """Row-major V projection for the chunk's 4 t-tiles."""
            for tl in range(4):
                ti = 4 * qc + tl
                ps = psp.tile([P, 512], F32, tag="qk")
                ps = ps[:, 0:256]
                for jp in range(4):
                    nc.tensor.matmul(
                        ps,
                        htr[:, qc, 2 * jp:2 * jp + 2, 128 * tl:128 * (tl + 1)],
                        wv[:, 2 * jp:2 * jp + 2, :],
                        start=(jp == 0), stop=(jp == 3), perf_mode=DR)
                vdst = v_t[ti // 2].rearrange(
                    "p (m h c) -> p m h c", m=2,
                    c=96)[:, ti % 2, :, 0:HEAD_DIM]
                vsrc = ps.rearrange("p (h c) -> p h c", c=HEAD_DIM)
                nc.vector.tensor_scalar_mul(vdst, vsrc, 1.0 / WS)

        def st_pass(qT, qc, h):
            """Scores^T + exp for head h / chunk qc -> pt pair tiles."""
            bp = 64 * (h % 2)
            blk = h // 2
            qs = qT[bp:bp + 64, 512 * blk:512 * (blk + 1)]
            pts = []
            for kp in range(2 * qc + 2):
                pt = ptp.tile([P, 1024], FP8)
                zp = max(0, 256 * kp - 512 * qc)
                sm = psp.tile([P, 1024], F32, tag="sm", bufs=2)
                for mem in range(2):
                    ki = 2 * kp + mem
                    kslice = kT[bp:bp + 64,
                                1024 * (ki // 4) + 512 * blk + 128 * (ki % 4):
                                1024 * (ki // 4) + 512 * blk + 128 * (ki % 4 + 1)]
                    # both members computed from the pair base zp (the odd
                    # member's leading 128 cols are real scores that the
                    # m256 mask below zeroes) so one strided exp covers
                    # the whole pair
                    nc.tensor.matmul(sm[:, 512 * mem + zp:512 * (mem + 1)],
                                     kslice, qs[:, zp:512],
                                     start=True, stop=True)
                w = 512 - zp
                smr = sm[:].rearrange("p (m n) -> p m n", n=512)[:, :, zp:512]
                ptr = pt[:].rearrange("p (m n) -> p m n", n=512)[:, :, zp:512]
                nc.scalar.activation(ptr, smr,
                                     mybir.ActivationFunctionType.Exp,
                                     scale=SC)
                if kp >= 2 * qc:   # diagonal pair: zero+tri masks
                    nc.vector.tensor_mul(pt[:, zp:zp + P],
                                         pt[:, zp:zp + P], tri)
                    nc.vector.tensor_mul(pt[:, 512 + zp:512 + zp + 256],
                                         pt[:, 512 + zp:512 + zp + 256],
                                         m256)
                pts.append((pt, zp))
            return pts

        def pv_pass(pts, g, att):
            """PV (fp8 DoubleRow) + softmax normalization for g=(qc,h)."""
            qc, h = g
            bp = 64 * (h % 2)
            blk = h // 2
            pvt = psp.tile([P, 512], F32, tag="pvop")
            pv = pvt[0:96, :]
            for kp, (pt, zp) in enumerate(pts):
                vw = v_t[kp].rearrange(
                    "p (m hc) -> p m hc", m=2)[:, :, 96 * h:96 * (h + 1)]
                pr = pt[:].rearrange("p (m n) -> p m n", m=2)[:, :, zp:512]
                nc.tensor.matmul(pv[:, zp:512], vw, pr,
                                 start=(kp == 0), stop=(kp == len(pts) - 1),
                                 perf_mode=DR)
            pv = pvt
            srow = nrm.tile([1, 512], F32, tag="srow")
            nc.vector.tensor_copy(srow[:], pvt[64:65, :])
            rrow = nrm.tile([1, 512], F32, tag="rrow")
            nc.vector.reciprocal_approx_fast(rrow[:], srow[:])
            bcast = nrm.tile([64, 512], F32, tag="bcast")
            nc.gpsimd.partition_broadcast(bcast[:], rrow[:])
            nc.vector.tensor_mul(
                att[bp:bp + 64, 512 * blk:512 * (blk + 1)],
                pvt[0:64, :], bcast[:])

        def outproj(qc, tl, att):
            """fp8 DoubleRow out-projection for one t-tile."""
            ar = att[:].rearrange("p (j q) -> p j q", j=2)
            ti = 4 * qc + tl
            o_t = orow.tile([P, D], BF16)
            for ec in range(2):
                op = psp.tile([P, 512], F32, tag="pvop")
                nc.tensor.matmul(op[:], ar[:, :, 128 * tl:128 * (tl + 1)],
                                 wo[:, :, 512 * ec:512 * (ec + 1)],
                                 start=True, stop=True, perf_mode=DR)
                if ec == 0:
                    nc.vector.tensor_scalar_mul(o_t[:, 0:512], op[:], 1.0 / WS)
                else:
                    nc.scalar.mul(o_t[:, 512:1024], op[:], 1.0 / WS)
            nc.sync.dma_start(out=outp[P * ti:P * (ti + 1), :], in_=o_t[:])

        # ---------------- emission: software-pipelined groups ----------
        # pv of group g-1 is emitted after the st/exp of group g so the PE
        # has dense PV work while ACT chews through group g's exps.  The
        # NEXT chunk's projections are interleaved into the current
        # chunk's later head iterations so the PE never dips at chunk
        # boundaries, and the previous chunk's out-proj tiles are spread
        # one per head iteration.
        prev = None
        att_prev = None
        qT = proj_qk(0, 0)
        proj_qk(0, 1, qT)
        proj_v(0)
        for qc in range(NQC):
            if DEBUG and qc == 0:
                nc.sync.dma_start(out=dbg["d_qT0"], in_=qT[:])
            att = atp.tile([P, 1024], FP8, tag="att")
            for h in range(HL):
                pts = st_pass(qT, qc, h)
                if DEBUG and qc == 0 and h == 0:
                    nc.sync.dma_start(out=dbg["d_pt00"], in_=pts[0][0][:])
                if prev is not None:
                    pv_pass(*prev)
                if qc > 0:
                    outproj(qc - 1, h, att_prev)
                    if DEBUG and qc == 1 and h == HL - 1:
                        nc.sync.dma_start(out=dbg["d_att0"], in_=att_prev[:])
                if qc + 1 < NQC:
                    if h == 2:
                        qT_next = proj_qk(qc + 1, 0)
                    elif h == 3:
                        proj_qk(qc + 1, 1, qT_next)
                        proj_v(qc + 1)
                prev = (pts, (qc, h), att)
            att_prev = att
            if qc + 1 < NQC:
                qT = qT_next
        pv_pass(*prev)
        for tl in range(4):
            outproj(NQC - 1, tl, att_prev)
        if DEBUG:
            nc.sync.dma_start(out=dbg["d_kT"], in_=kT[:])
            nc.sync.dma_start(out=dbg["d_v0"], in_=v_t[0][:])


# ---------------- host-side driver ----------------

_CACHE = {}


def _get_program():
    if "nc" not in _CACHE:
        _CACHE["nc"] = _build_program()
    return _CACHE["nc"]


def _rope_tables():
    half = HEAD_DIM // 2
    inv_freq = (1.0 / (ROPE_BASE ** (np.arange(half, dtype=np.float32) / half))
                ).astype(np.float32)
    pos = np.arange(T, dtype=np.float32)
    freqs = pos[:, None] * inv_freq[None, :]
    emb = np.concatenate([freqs, freqs], axis=-1).astype(np.float32)
    return np.cos(emb), np.sin(emb)


def make_in_maps(x, norm_w, w_qkv, w_out):
    f8 = ml_dtypes.float8_e4m3
    bf = ml_dtypes.bfloat16
    # RoPE pair-interleave: partition 2i <- dh i, partition 2i+1 <- dh i+32
    perm = np.empty(HEAD_DIM, dtype=np.int64)
    perm[0::2] = np.arange(32)
    perm[1::2] = np.arange(32) + 32
    sgn = np.where(perm < 32, -1.0, 1.0).astype(np.float32)  # rotate-half sign

    cos, sin = _rope_tables()          # (T, 64)
    cs_pack = np.empty((P, CS_COLS), dtype=np.float32)
    cs_pack[:, 0:T] = np.tile(cos.T[perm] / WS, (2, 1))
    cs_pack[:, T:2 * T] = np.tile(sin.T[perm] * sgn[:, None] / WS, (2, 1))
    cs_pack = cs_pack.astype(bf)

    tri = (np.arange(P)[None, :] >= np.arange(P)[:, None]).astype(np.float32)

    w_fold = (w_qkv * norm_w[None, :]) * WS   # (3D, D)
    rinv = 1.0 / np.sqrt((x ** 2).mean(axis=-1, keepdims=True) + EPS)
    h = (x * rinv).astype(np.float32)         # (B, T, D)

    in_maps = []
    for c in range(NCORES):
        b, hg = c // 4, c % 4
        # ht: [p, qc x j x t] = h[b, 512qc+t, 128j+p]
        ht_pack = np.ascontiguousarray(
            h[b].reshape(NQC, 512, 8, P).transpose(3, 0, 2, 1)
            .reshape(P, NQC * 4096)).astype(f8)

        w8_pack = np.empty((P, W8_COLS), dtype=np.float32)
        # wqk: [p, j x m x c] = Wfold[row(m,c), 128j+p]
        cidx = np.arange(P)
        dh_perm = perm[cidx % HEAD_DIM]            # c -> dh
        for m in range(4):
            if m < 2:
                rows = 256 * hg + HEAD_DIM * (2 * m + cidx // HEAD_DIM) \
                    + dh_perm
            else:
                rows = D + 256 * hg \
                    + HEAD_DIM * (2 * (m - 2) + cidx // HEAD_DIM) + dh_perm
            blk = w_fold[rows, :]                  # (128c, D)
            for j in range(8):
                w8_pack[:, W8_QK + 1024 * m + 128 * j:
                        W8_QK + 1024 * m + 128 * (j + 1)] = \
                    blk[:, 128 * j:128 * (j + 1)].T
        # wv: [p, j x c] = Wfold[2D + 256hg + c, 128j+p]
        vrows = 2 * D + 256 * hg + np.arange(256)
        vblk = w_fold[vrows, :]                    # (256, D)
        for j in range(8):
            w8_pack[:, W8_V + 256 * j:W8_V + 256 * (j + 1)] = \
                vblk[:, 128 * j:128 * (j + 1)].T
        # wo: [p, jb x e] = w_out[e, 256hg + 128jb + p] * WS
        for jb in range(2):
            w8_pack[:, W8_WO + D * jb:W8_WO + D * (jb + 1)] = \
                w_out[:, 256 * hg + 128 * jb:256 * hg + 128 * (jb + 1)].T * WS
        w8_pack[:, W8_TRI:W8_TRI + P] = tri
        w8_pack[:, W8_M256:W8_M256 + P] = 0.0
        w8_pack[:, W8_M256 + P:W8_M256 + 256] = tri
        in_maps.append({
            "w8": w8_pack.astype(f8),
            "cs": cs_pack,
            "ht": ht_pack,
        })
    return in_maps


def assemble(x, results):
    out = np.empty((B, T, D), dtype=np.float32)
    for b in range(B):
        acc = x[b].astype(np.float32).copy()
        for hg in range(4):
            acc += results[4 * b + hg]["outp"].astype(np.float32)
        out[b] = acc
    return out


def kernel(x, norm_w, w_qkv, w_out, trace=False):
    x = np.asarray(x, dtype=np.float32)
    norm_w = np.asarray(norm_w, dtype=np.float32)
    w_qkv = np.asarray(w_qkv, dtype=np.float32)
    w_out = np.asarray(w_out, dtype=np.float32)
    nc = _get_program()
    in_maps = make_in_maps(x, norm_w, w_qkv, w_out)
    res = run_bass_kernel_spmd(nc, in_maps, core_ids=list(range(NCORES)),
                               trace=trace)
    _CACHE["last_results"] = res
    return assemble(x, res.results)


# revision 24
# speedup vs baseline: 1.2165x; 1.2165x over previous
"""Causal self-attention block (RMSNorm + QKV + RoPE + causal attention +
out-proj + residual) on 8 Trainium2 NeuronCores.

Sharding: batch (B=2) x head-groups (16 heads -> 4 groups of 4) = 8 shards.
Core c handles batch b = c // 4 and heads [4*(c%4), 4*(c%4)+4).  The host
sums the 4 partial out-projections per batch and adds the residual.

v2 design (vs the row-major v1):
 - RMSNorm is folded host-side into the activations (h = x * rinv), and
   norm_w into w_qkv, so the device sees pre-normalized hT and does no
   stats / scaling work at all.
 - Q/K/V are projected DIRECTLY in transposed layout (dh on partitions,
   t free) by making the weight block the stationary operand, eliminating
   all PE transposes of v1.
 - RoPE pairs (i, i+32) are host-interleaved to adjacent partitions so
   rotate-half becomes a single DVE stream_shuffle (mask swaps even/odd
   partitions within each 32-partition quadrant).
 - fp8(e4m3) + MatmulPerfMode.DoubleRow (2 rows/cycle) for the QKV
   projection, PV, and out-projection matmuls; scores stay bf16 (exp
   input precision).  Weights are pre-scaled by WS=32 host-side to clear
   the fp8 denormal range; 1/WS is folded into the RoPE tables / evicts.
 - All inputs are host-packed into their exact SBUF layouts so each DMA
   moves large contiguous rows (128 descriptors per tensor).
 - scores^T (k on partitions) + ones-column in V give softmax denominators
   from the PV matmul; no max-subtraction needed (scores are O(3)).
"""

import numpy as np

import ml_dtypes

import concourse.bacc as bacc
import concourse.tile as tile
from concourse import mybir
from concourse.bass_utils import run_bass_kernel_spmd

# Problem shapes (hardcoded per contract)
B, T, D, NHEADS = 2, 2048, 1024, 16
HEAD_DIM = 64
EPS = 1e-6
ROPE_BASE = 10000.0

HL = 4            # heads per core
P = 128
NT = T // P       # 16 t-tiles
NQC = T // 512    # 4 query chunks
NCORES = 8
WS = 32.0         # host-side weight pre-scale (fp8 denormal avoidance)
SC = 0.125        # 1/sqrt(64)

F32 = mybir.dt.float32
BF16 = mybir.dt.bfloat16
FP8 = mybir.dt.float8e4
DR = mybir.MatmulPerfMode.DoubleRow

# w8 column map (fp8 constants, packed host-side in exact SBUF layout)
W8_QK = 0          # 4096 cols: [m(4) x j(8) x c(128)]
W8_V = 4096        # 2048 cols: [j(8) x c(256)]
W8_WO = 6144       # 2048 cols: [jb(2) x e(1024)]
W8_TRI = 8192      # 128 cols
W8_M256 = 8320     # 256 cols: [zeros(128) | tri] for odd diagonal members
W8_COLS = 8576

CS_COLS = 4096     # cos (2048) | sin (2048), bf16

# stream-shuffle mask: swap even/odd partitions within each 32-quadrant
SHUF = [i ^ 1 for i in range(32)]


def _build_program():
    nc = bacc.Bacc("TRN2", target_bir_lowering=False, debug=False,
                   num_devices=NCORES)

    w8 = nc.dram_tensor("w8", [P, W8_COLS], FP8, kind="ExternalInput").ap()
    cs = nc.dram_tensor("cs", [P, CS_COLS], BF16, kind="ExternalInput").ap()
    ht = nc.dram_tensor("ht", [P, NQC * 4096], FP8, kind="ExternalInput").ap()
    outp = nc.dram_tensor("outp", [T, D], BF16, kind="ExternalOutput").ap()
    dbg = {}
    if DEBUG:
        dbg["d_qT0"] = nc.dram_tensor("d_qT0", [P, 1024], BF16,
                                      kind="ExternalOutput").ap()
        dbg["d_kT"] = nc.dram_tensor("d_kT", [P, NQC * 1024], BF16,
                                     kind="ExternalOutput").ap()
        dbg["d_v0"] = nc.dram_tensor("d_v0", [P, 768], FP8,
                                     kind="ExternalOutput").ap()
        dbg["d_att0"] = nc.dram_tensor("d_att0", [P, 1024], FP8,
                                       kind="ExternalOutput").ap()
        dbg["d_pt00"] = nc.dram_tensor("d_pt00", [P, 1024], FP8,
                                       kind="ExternalOutput").ap()

    with tile.TileContext(nc) as tc:
        _emit(tc, w8, cs, ht, outp, dbg)

    nc.compile()
    return nc


DEBUG = False


def _emit(tc, w8, cs, ht, outp, dbg=None):
    nc = tc.nc
    from contextlib import ExitStack
    ctx = ExitStack()
    with ctx:
        const = ctx.enter_context(tc.tile_pool(name="const", bufs=1))
        persist = ctx.enter_context(tc.tile_pool(name="persist", bufs=1))
        qtp = ctx.enter_context(tc.tile_pool(name="qtp", bufs=2))
        atp = ctx.enter_context(tc.tile_pool(name="atp", bufs=2))
        ptp = ctx.enter_context(tc.tile_pool(name="ptp", bufs=20))
        shp = ctx.enter_context(tc.tile_pool(name="shp", bufs=3))
        s2p = ctx.enter_context(tc.tile_pool(name="s2p", bufs=3))
        nrm = ctx.enter_context(tc.tile_pool(name="nrm", bufs=4))
        orow = ctx.enter_context(tc.tile_pool(name="orow", bufs=3))
        # PSUM budget (8 banks): qk 2 + sm 2x2 + pvop 2
        psp = ctx.enter_context(
            tc.tile_pool(name="psp", bufs=2, space="PSUM"))

        # ---- SBUF-resident inputs (host-packed layouts) ----
        # split across the two HWDGE queues (sync + scalar) so the first
        # projection's operands land as early as possible
        w8_sb = const.tile([P, W8_COLS], FP8)
        for m in range(4):
            nc.sync.dma_start(out=w8_sb[:, 1024 * m:1024 * (m + 1)],
                              in_=w8[:, 1024 * m:1024 * (m + 1)])
        ht_sb = persist.tile([P, NQC * 4096], FP8)
        nc.scalar.dma_start(out=ht_sb[:, 0:2048], in_=ht[:, 0:2048])
        nc.sync.dma_start(out=ht_sb[:, 2048:4096], in_=ht[:, 2048:4096])
        cs_sb = const.tile([P, CS_COLS], BF16)
        nc.sync.dma_start(out=w8_sb[:, 4096:W8_COLS], in_=w8[:, 4096:W8_COLS])
        nc.scalar.dma_start(out=cs_sb[:], in_=cs[:])
        for qc in range(1, NQC):
            eng = nc.sync if qc % 2 else nc.scalar
            eng.dma_start(out=ht_sb[:, 4096 * qc:4096 * (qc + 1)],
                          in_=ht[:, 4096 * qc:4096 * (qc + 1)])

        wqk = w8_sb[:, W8_QK:W8_QK + 4096].rearrange(
            "p (m j c) -> p m j c", m=4, c=P)
        wv = w8_sb[:, W8_V:W8_V + 2048].rearrange("p (j c) -> p j c", c=256)
        wo = w8_sb[:, W8_WO:W8_WO + 2048].rearrange("p (j e) -> p j e", e=D)
        tri = w8_sb[:, W8_TRI:W8_TRI + P]
        m256 = w8_sb[:, W8_M256:W8_M256 + 256]
        htr = ht_sb[:].rearrange("p (q j t) -> p q j t", q=NQC, t=512)

        # K^T persistent: chunk qc block b (heads 2b,2b+1) at cols
        # [1024*qc + 512*b]; partitions = RoPE-interleaved dh of 2 heads.
        kT = persist.tile([P, NQC * 1024], BF16)
        # V row-major pair tiles: pair kp = k-tiles (2kp, 2kp+1); layout
        # [p, member(2) x head(4) x c(96)]; c=64 is the ones column and
        # c=65..95 zero padding (dual-fp8 ldweights needs M % 32 == 0).
        v_t = [persist.tile([P, 768], FP8, name=f"v{i}", tag=f"v{i}")
               for i in range(NT // 2)]
        for kp in range(NT // 2):
            vr = v_t[kp].rearrange("p (m h c) -> p m h c", m=2, c=96)
            nc.vector.memset(vr[:, :, :, HEAD_DIM:HEAD_DIM + 1], 1.0)
            nc.vector.memset(vr[:, :, :, HEAD_DIM + 1:], 0.0)

        def rope_evict(ps, dst, qc):
            """dst = ps*cos + shuffle(ps)*sin, all (128, 512); 1/WS folded
            into the host tables."""
            ct = cs_sb[:, 512 * qc:512 * (qc + 1)]
            st = cs_sb[:, 2048 + 512 * qc:2048 + 512 * (qc + 1)]
            t1 = shp.tile([P, 512], F32, tag="t1")
            nc.vector.stream_shuffle(t1[:], ps, SHUF)
            t2 = s2p.tile([P, 512], BF16, tag="t2")
            nc.vector.tensor_mul(t2[:], t1[:], st)
            nc.vector.tensor_mul(dst, ps, ct)
            nc.vector.tensor_add(dst, dst, t2[:])

        def proj_qk(qc, part, qT=None):
            """Transposed-domain Q/K projection + RoPE for one 512-chunk.
            part 0 = the two q blocks (allocates qT); part 1 = k blocks."""
            if part == 0:
                qT = qtp.tile([P, 1024], BF16, tag="qT")
            for m in (0, 1) if part == 0 else (2, 3):
                ps = psp.tile([P, 512], F32, tag="qk")
                for jp in range(4):
                    nc.tensor.matmul(
                        ps[:], wqk[:, m, 2 * jp:2 * jp + 2, :],
                        htr[:, qc, 2 * jp:2 * jp + 2, :],
                        start=(jp == 0), stop=(jp == 3), perf_mode=DR)
                if m < 2:
                    dst = qT[:, 512 * m:512 * (m + 1)]
                else:
                    dst = kT[:, 1024 * qc + 512 * (m - 2):
                             1024 * qc + 512 * (m - 1)]
                rope_evict(ps[:], dst, qc)
            return qT

        def proj_v(qc):
            """Row-major V projection for the chunk's 4 t-tiles."""
            for tl in range(4):
                ti = 4 * qc + tl
                ps = psp.tile([P, 512], F32, tag="qk")
                ps = ps[:, 0:256]
                for jp in range(4):
                    nc.tensor.matmul(
                        ps,
                        htr[:, qc, 2 * jp:2 * jp + 2, 128 * tl:128 * (tl + 1)],
                        wv[:, 2 * jp:2 * jp + 2, :],
                        start=(jp == 0), stop=(jp == 3), perf_mode=DR)
                vdst = v_t[ti // 2].rearrange(
                    "p (m h c) -> p m h c", m=2,
                    c=96)[:, ti % 2, :, 0:HEAD_DIM]
                vsrc = ps.rearrange("p (h c) -> p h c", c=HEAD_DIM)
                nc.vector.tensor_scalar_mul(vdst, vsrc, 1.0 / WS)

        def st_pass(qT, qc, h, prev=None):
            """Scores^T + exp for head h / chunk qc -> pt pair tiles.

            When ``prev`` is given, the previous group's PV matmuls are
            interleaved between this group's st pairs so the in-order PE
            queue always has ready work while ACT catches up on exps (the
            previous group's exps are long done).  The previous group's
            softmax normalization is emitted at the end."""
            bp = 64 * (h % 2)
            blk = h // 2
            qs = qT[bp:bp + 64, 512 * blk:512 * (blk + 1)]
            if prev is not None:
                ppts, (pqc, ph), patt = prev
                pvt = psp.tile([P, 512], F32, tag="pvop")

                def pv_emit(kp):
                    pt, zp = ppts[kp]
                    vw = v_t[kp].rearrange(
                        "p (m hc) -> p m hc", m=2)[:, :, 96 * ph:96 * (ph + 1)]
                    pr = pt[:].rearrange("p (m n) -> p m n",
                                         m=2)[:, :, zp:512]
                    nc.tensor.matmul(pvt[0:96, zp:512], vw, pr,
                                     start=(kp == 0),
                                     stop=(kp == len(ppts) - 1),
                                     perf_mode=DR)
            pts = []
            npairs = 2 * qc + 2
            for kp in range(npairs):
                pt = ptp.tile([P, 1024], FP8)
                zp = max(0, 256 * kp - 512 * qc)
                sm = psp.tile([P, 1024], F32, tag="sm", bufs=2)
                for mem in range(2):
                    ki = 2 * kp + mem
                    kslice = kT[bp:bp + 64,
                                1024 * (ki // 4) + 512 * blk + 128 * (ki % 4):
                                1024 * (ki // 4) + 512 * blk + 128 * (ki % 4 + 1)]
                    # both members computed from the pair base zp (the odd
                    # member's leading 128 cols are real scores that the
                    # m256 mask below zeroes) so one strided exp covers
                    # the whole pair
                    nc.tensor.matmul(sm[:, 512 * mem + zp:512 * (mem + 1)],
                                     kslice, qs[:, zp:512],
                                     start=True, stop=True)
                if prev is not None and kp < len(ppts):
                    pv_emit(kp)
                smr = sm[:].rearrange("p (m n) -> p m n", n=512)[:, :, zp:512]
                ptr = pt[:].rearrange("p (m n) -> p m n", n=512)[:, :, zp:512]
                nc.scalar.activation(ptr, smr,
                                     mybir.ActivationFunctionType.Exp,
                                     scale=SC)
                if kp >= 2 * qc:   # diagonal pair: zero+tri masks
                    nc.vector.tensor_mul(pt[:, zp:zp + P],
                                         pt[:, zp:zp + P], tri)
                    nc.vector.tensor_mul(pt[:, 512 + zp:512 + zp + 256],
                                         pt[:, 512 + zp:512 + zp + 256],
                                         m256)
                pts.append((pt, zp))
            if prev is not None:
                for kp in range(npairs, len(ppts)):
                    pv_emit(kp)
                bpp = 64 * (ph % 2)
                blkp = ph // 2
                srow = nrm.tile([1, 512], F32, tag="srow")
                nc.vector.tensor_copy(srow[:], pvt[64:65, :])
                rrow = nrm.tile([1, 512], F32, tag="rrow")
                nc.vector.reciprocal_approx_fast(rrow[:], srow[:])
                bcast = nrm.tile([64, 512], F32, tag="bcast")
                nc.gpsimd.partition_broadcast(bcast[:], rrow[:])
                nc.vector.tensor_mul(
                    patt[bpp:bpp + 64, 512 * blkp:512 * (blkp + 1)],
                    pvt[0:64, :], bcast[:])
            return pts

        def pv_pass(pts, g, att):
            """PV (fp8 DoubleRow) + softmax normalization for g=(qc,h)."""
            qc, h = g
            bp = 64 * (h % 2)
            blk = h // 2
            pvt = psp.tile([P, 512], F32, tag="pvop")
            pv = pvt[0:96, :]
            for kp, (pt, zp) in enumerate(pts):
                vw = v_t[kp].rearrange(
                    "p (m hc) -> p m hc", m=2)[:, :, 96 * h:96 * (h + 1)]
                pr = pt[:].rearrange("p (m n) -> p m n", m=2)[:, :, zp:512]
                nc.tensor.matmul(pv[:, zp:512], vw, pr,
                                 start=(kp == 0), stop=(kp == len(pts) - 1),
                                 perf_mode=DR)
            pv = pvt
            srow = nrm.tile([1, 512], F32, tag="srow")
            nc.vector.tensor_copy(srow[:], pvt[64:65, :])
            rrow = nrm.tile([1, 512], F32, tag="rrow")
            nc.vector.reciprocal_approx_fast(rrow[:], srow[:])
            bcast = nrm.tile([64, 512], F32, tag="bcast")
            nc.gpsimd.partition_broadcast(bcast[:], rrow[:])
            nc.vector.tensor_mul(
                att[bp:bp + 64, 512 * blk:512 * (blk + 1)],
                pvt[0:64, :], bcast[:])

        def outproj(qc, tl, att):
            """fp8 DoubleRow out-projection for one t-tile."""
            ar = att[:].rearrange("p (j q) -> p j q", j=2)
            ti = 4 * qc + tl
            o_t = orow.tile([P, D], BF16)
            for ec in range(2):
                op = psp.tile([P, 512], F32, tag="pvop")
                nc.tensor.matmul(op[:], ar[:, :, 128 * tl:128 * (tl + 1)],
                                 wo[:, :, 512 * ec:512 * (ec + 1)],
                                 start=True, stop=True, perf_mode=DR)
                if ec == 0:
                    nc.vector.tensor_scalar_mul(o_t[:, 0:512], op[:], 1.0 / WS)
                else:
                    nc.scalar.mul(o_t[:, 512:1024], op[:], 1.0 / WS)
            nc.sync.dma_start(out=outp[P * ti:P * (ti + 1), :], in_=o_t[:])

        # ---------------- emission: software-pipelined groups ----------
        # pv of group g-1 is emitted after the st/exp of group g so the PE
        # has dense PV work while ACT chews through group g's exps.  The
        # NEXT chunk's projections are interleaved into the current
        # chunk's later head iterations so the PE never dips at chunk
        # boundaries, and the previous chunk's out-proj tiles are spread
        # one per head iteration.
        prev = None
        att_prev = None
        qT = proj_qk(0, 0)
        proj_qk(0, 1, qT)
        proj_v(0)
        for qc in range(NQC):
            if DEBUG and qc == 0:
                nc.sync.dma_start(out=dbg["d_qT0"], in_=qT[:])
            att = atp.tile([P, 1024], FP8, tag="att")
            for h in range(HL):
                pts = st_pass(qT, qc, h, prev)
                if DEBUG and qc == 0 and h == 0:
                    nc.sync.dma_start(out=dbg["d_pt00"], in_=pts[0][0][:])
                if qc > 0:
                    outproj(qc - 1, h, att_prev)
                    if DEBUG and qc == 1 and h == HL - 1:
                        nc.sync.dma_start(out=dbg["d_att0"], in_=att_prev[:])
                if qc + 1 < NQC:
                    if h == 2:
                        qT_next = proj_qk(qc + 1, 0)
                    elif h == 3:
                        proj_qk(qc + 1, 1, qT_next)
                        proj_v(qc + 1)
                prev = (pts, (qc, h), att)
            att_prev = att
            if qc + 1 < NQC:
                qT = qT_next
        pv_pass(*prev)
        for tl in range(4):
            outproj(NQC - 1, tl, att_prev)
        if DEBUG:
            nc.sync.dma_start(out=dbg["d_kT"], in_=kT[:])
            nc.sync.dma_start(out=dbg["d_v0"], in_=v_t[0][:])


# ---------------- host-side driver ----------------

_CACHE = {}


def _get_program():
    if "nc" not in _CACHE:
        _CACHE["nc"] = _build_program()
    return _CACHE["nc"]


def _rope_tables():
    half = HEAD_DIM // 2
    inv_freq = (1.0 / (ROPE_BASE ** (np.arange(half, dtype=np.float32) / half))
                ).astype(np.float32)
    pos = np.arange(T, dtype=np.float32)
    freqs = pos[:, None] * inv_freq[None, :]
    emb = np.concatenate([freqs, freqs], axis=-1).astype(np.float32)
    return np.cos(emb), np.sin(emb)


def make_in_maps(x, norm_w, w_qkv, w_out):
    f8 = ml_dtypes.float8_e4m3
    bf = ml_dtypes.bfloat16
    # RoPE pair-interleave: partition 2i <- dh i, partition 2i+1 <- dh i+32
    perm = np.empty(HEAD_DIM, dtype=np.int64)
    perm[0::2] = np.arange(32)
    perm[1::2] = np.arange(32) + 32
    sgn = np.where(perm < 32, -1.0, 1.0).astype(np.float32)  # rotate-half sign

    cos, sin = _rope_tables()          # (T, 64)
    cs_pack = np.empty((P, CS_COLS), dtype=np.float32)
    cs_pack[:, 0:T] = np.tile(cos.T[perm] / WS, (2, 1))
    cs_pack[:, T:2 * T] = np.tile(sin.T[perm] * sgn[:, None] / WS, (2, 1))
    cs_pack = cs_pack.astype(bf)

    tri = (np.arange(P)[None, :] >= np.arange(P)[:, None]).astype(np.float32)

    w_fold = (w_qkv * norm_w[None, :]) * WS   # (3D, D)
    rinv = 1.0 / np.sqrt((x ** 2).mean(axis=-1, keepdims=True) + EPS)
    h = (x * rinv).astype(np.float32)         # (B, T, D)

    in_maps = []
    for c in range(NCORES):
        b, hg = c // 4, c % 4
        # ht: [p, qc x j x t] = h[b, 512qc+t, 128j+p]
        ht_pack = np.ascontiguousarray(
            h[b].reshape(NQC, 512, 8, P).transpose(3, 0, 2, 1)
            .reshape(P, NQC * 4096)).astype(f8)

        w8_pack = np.empty((P, W8_COLS), dtype=np.float32)
        # wqk: [p, j x m x c] = Wfold[row(m,c), 128j+p]
        cidx = np.arange(P)
        dh_perm = perm[cidx % HEAD_DIM]            # c -> dh
        for m in range(4):
            if m < 2:
                rows = 256 * hg + HEAD_DIM * (2 * m + cidx // HEAD_DIM) \
                    + dh_perm
            else:
                rows = D + 256 * hg \
                    + HEAD_DIM * (2 * (m - 2) + cidx // HEAD_DIM) + dh_perm
            blk = w_fold[rows, :]                  # (128c, D)
            for j in range(8):
                w8_pack[:, W8_QK + 1024 * m + 128 * j:
                        W8_QK + 1024 * m + 128 * (j + 1)] = \
                    blk[:, 128 * j:128 * (j + 1)].T
        # wv: [p, j x c] = Wfold[2D + 256hg + c, 128j+p]
        vrows = 2 * D + 256 * hg + np.arange(256)
        vblk = w_fold[vrows, :]                    # (256, D)
        for j in range(8):
            w8_pack[:, W8_V + 256 * j:W8_V + 256 * (j + 1)] = \
                vblk[:, 128 * j:128 * (j + 1)].T
        # wo: [p, jb x e] = w_out[e, 256hg + 128jb + p] * WS
        for jb in range(2):
            w8_pack[:, W8_WO + D * jb:W8_WO + D * (jb + 1)] = \
                w_out[:, 256 * hg + 128 * jb:256 * hg + 128 * (jb + 1)].T * WS
        w8_pack[:, W8_TRI:W8_TRI + P] = tri
        w8_pack[:, W8_M256:W8_M256 + P] = 0.0
        w8_pack[:, W8_M256 + P:W8_M256 + 256] = tri
        in_maps.append({
            "w8": w8_pack.astype(f8),
            "cs": cs_pack,
            "ht": ht_pack,
        })
    return in_maps


def assemble(x, results):
    out = np.empty((B, T, D), dtype=np.float32)
    for b in range(B):
        acc = x[b].astype(np.float32).copy()
        for hg in range(4):
            acc += results[4 * b + hg]["outp"].astype(np.float32)
        out[b] = acc
    return out


def kernel(x, norm_w, w_qkv, w_out, trace=False):
    x = np.asarray(x, dtype=np.float32)
    norm_w = np.asarray(norm_w, dtype=np.float32)
    w_qkv = np.asarray(w_qkv, dtype=np.float32)
    w_out = np.asarray(w_out, dtype=np.float32)
    nc = _get_program()
    in_maps = make_in_maps(x, norm_w, w_qkv, w_out)
    res = run_bass_kernel_spmd(nc, in_maps, core_ids=list(range(NCORES)),
                               trace=trace)
    _CACHE["last_results"] = res
    return assemble(x, res.results)
